# revision 16
# baseline (speedup 1.0000x reference)
"""Trainium2 Bass kernel for nn_CausalRecurrenceLayer.

Sharding: 8 cores = 4 batches x 2 sequence-halves. Channel-major [c, t]
device layout for conv/gates/scan; output projection emits [t, j].

Single-pass design (no DRAM spills): phase A computes conv -> fp8 gate
matmuls (DoubleRow) -> decay am1 (f16, SBUF-resident) and gated input bb
(f16, SBUF-resident) -> local pass-1 scan (only the tail column is kept).
A 4KB AllGather exchanges h_last across sequence-half pairs. Phase B
re-runs the true scan from the received initial state straight out of
SBUF, then out-projection (f16 PE) + RMSNorm.

Self-contained: hardcodes shapes B=4, L=4096, d=1024.
"""
import sys

sys.path.insert(0, "/opt/trn_rl_repo")

import numpy as np
import ml_dtypes

import concourse.bass as bass  # noqa: F401
from concourse.bass import _add_dep_helper
import concourse.tile as tile
from concourse import bacc, mybir
from concourse import bass_utils

F32 = mybir.dt.float32
F32R = mybir.dt.float32r
F16 = mybir.dt.float16
F8 = mybir.dt.float8e4
AF = mybir.ActivationFunctionType
OP = mybir.AluOpType
DR = mybir.MatmulPerfMode.DoubleRow

B, L, D = 4, 4096, 1024
TH = L // 2      # per-core sequence extent
TT = 512         # time tile
NT = TH // TT    # 4
P = 128
CB = D // P      # 8 channel blocks
EPS = 1e-6
XS = 64.0        # fp8 scale on x_conv
WS = 16.0        # fp8 scale on gate weights
GS = 0.5 / (XS * WS)   # tanh pre-scale compensating both

_compiled = {}


def _build():
    nc = bacc.Bacc("TRN2", target_bir_lowering=False, debug=False, num_devices=8)

    x_d = nc.dram_tensor("x_sh", [D, TH + 3], F32R, kind="ExternalInput").ap()
    dw_d = nc.dram_tensor("dwk", [D, 4 * P], F32R, kind="ExternalInput").ap()
    wr_d = nc.dram_tensor("wr8", [D // 2, 2 * D], F8, kind="ExternalInput").ap()
    wi_d = nc.dram_tensor("wi8", [D // 2, 2 * D], F8, kind="ExternalInput").ap()
    wo_d = nc.dram_tensor("woT", [D, D], F16, kind="ExternalInput").ap()
    br_d = nc.dram_tensor("br_c", [P, CB], F32, kind="ExternalInput").ap()   # b_r/2
    bi_d = nc.dram_tensor("bi_c", [P, CB], F32, kind="ExternalInput").ap()   # b_i/2
    cb_d = nc.dram_tensor("cb_c", [P, CB], F32, kind="ExternalInput").ap()   # conv bias
    c1_d = nc.dram_tensor("c1_c", [P, CB], F32, kind="ExternalInput").ap()   # 4*ln(a_base)
    tm_d = nc.dram_tensor("tmask", [P, 1], F32, kind="ExternalInput").ap()
    y_d = nc.dram_tensor("y", [TH, D], F32, kind="ExternalOutput").ap()

    last_act = [None]
    _CHAINED = (AF.Tanh, AF.Exp, AF.Sqrt)

    def act(out, in_, func, **kw):
        ins = nc.scalar.activation(out, in_, func, **kw)
        if func in _CHAINED:
            if last_act[0] is not None:
                _add_dep_helper(ins.ins, last_act[0].ins, reason="act table order")
            last_act[0] = ins
        return ins

    with tile.TileContext(nc) as tc:
        with (
            tc.tile_pool(name="wpool", bufs=1) as wpool,
            tc.tile_pool(name="store", bufs=1) as store,
            tc.tile_pool(name="dram", bufs=1, space="DRAM") as dp,
        ):
            # ---- resident weights / constants ----
            br_t = wpool.tile([P, CB], F32, tag="br")
            nc.sync.dma_start(br_t[:], br_d)
            bi_t = wpool.tile([P, CB], F32, tag="bi")
            nc.sync.dma_start(bi_t[:], bi_d)
            cb_t = wpool.tile([P, CB], F32, tag="cbias")
            nc.sync.dma_start(cb_t[:], cb_d)
            c1_t = wpool.tile([P, CB], F32, tag="c1")
            nc.sync.dma_start(c1_t[:], c1_d)
            tm_t = wpool.tile([P, 1], F32, tag="tm")
            nc.sync.dma_start(tm_t[:], tm_d)
            dw_t, wr_t, wi_t, wo_t = [], [], [], []
            for cb in range(CB):
                t = wpool.tile([P, 4 * P], F32R, tag=f"dw{cb}", name=f"dw{cb}")
                nc.sync.dma_start(t[:], dw_d[cb * P:(cb + 1) * P, :])
                dw_t.append(t)
            for kbp in range(CB // 2):
                t = wpool.tile([P, 2 * D], F8, tag=f"wr{kbp}", name=f"wr{kbp}")
                nc.sync.dma_start(t[:], wr_d[kbp * P:(kbp + 1) * P, :])
                wr_t.append(t)
                t = wpool.tile([P, 2 * D], F8, tag=f"wi{kbp}", name=f"wi{kbp}")
                nc.sync.dma_start(t[:], wi_d[kbp * P:(kbp + 1) * P, :])
                wi_t.append(t)
            for cb in range(CB):
                t = wpool.tile([P, D], F16, tag=f"wo{cb}", name=f"wo{cb}")
                nc.sync.dma_start(t[:], wo_d[cb * P:(cb + 1) * P, :])
                wo_t.append(t)
            eps_t = wpool.tile([P, 1], F32, tag="eps")
            nc.vector.memset(eps_t[:], EPS)
            qtr_t = wpool.tile([P, 1], F32, tag="qtr")
            nc.vector.memset(qtr_t[:], 0.25)
            xsb_t = wpool.tile([P, TT], F16, tag="xsb")
            nc.vector.memset(xsb_t[:], XS)

            # SBUF residents carried from phase A to phase B
            am1_st = [wpool.tile([P, TH], F16, tag=f"am1_{cb}", name=f"am1_{cb}")
                      for cb in range(CB)]
            bb_st = [wpool.tile([P, TH], F16, tag=f"bb_{cb}", name=f"bb_{cb}")
                     for cb in range(CB)]

            hl_sb = store.tile([P, CB], F32, tag="hl")
            cy_t = [store.tile([P, 1], F32, tag=f"cy{cb}", name=f"cy{cb}")
                    for cb in range(CB)]
            g0 = store.tile([P, CB], F32, tag="g0")
            init_c = store.tile([P, CB], F32, tag="init_c")
            ag_in = dp.tile([1, D], F32, tag="ag_in")
            ag_out = dp.tile([2, D], F32, tag="ag_out")

            # =========== PHASE A ===========
            with (
                tc.tile_pool(name="sbufA", bufs=1) as sa,
                tc.tile_pool(name="psumA", bufs=1, space="PSUM") as psa,
            ):
                def conv_loop(t0):
                    # conv on PE + DVE evict + Pool fp8 cast
                    xc_t, xc8_t = [], []
                    for cb in range(CB):
                        xt = sa.tile([P, TT + 3], F32R, tag="xraw", bufs=2)
                        nc.sync.dma_start(
                            xt[:], x_d[cb * P:(cb + 1) * P, t0 * TT:t0 * TT + TT + 3])
                        xc_ps = psa.tile([P, TT], F32, tag="xc_ps", bufs=3)
                        for k in range(4):
                            nc.tensor.matmul(xc_ps[:], dw_t[cb][:, k * P:(k + 1) * P],
                                             xt[:, k:k + TT], start=(k == 0), stop=(k == 3))
                        xc = sa.tile([P, TT], F16, tag="xc", bufs=18)
                        nc.vector.tensor_scalar_add(xc[:], xc_ps[:], cb_t[:, cb:cb + 1])
                        xc_t.append(xc)
                        if cb % 2 == 0:
                            x8 = sa.tile([P, 2 * TT], F8, tag="xc8", bufs=8)
                            xc8_t.append(x8)
                        nc.gpsimd.tensor_tensor(
                            xc8_t[cb // 2][:, (cb % 2) * TT:(cb % 2 + 1) * TT],
                            xc[:], xsb_t[:], OP.mult)
                    return xc_t, xc8_t

                xc_cur, xc8_cur = conv_loop(0)
                for t0 in range(NT):
                    xc_nxt = xc8_nxt = None
                    if t0 + 1 < NT:
                        xc_nxt, xc8_nxt = conv_loop(t0 + 1)

                    # -- fp8 DoubleRow gate matmuls + tanh/exp (one table set) --
                    a_tiles = [None] * CB
                    thi_tiles = [None] * CB
                    for cb in range(CB):
                        r_ps = psa.tile([P, TT], F32, tag="r_ps", bufs=2)
                        i_ps = psa.tile([P, TT], F32, tag="i_ps", bufs=2)
                        for kbp in range(CB // 2):
                            nc.tensor.matmul(
                                r_ps[:],
                                wr_t[kbp][:].rearrange("p (k m) -> p k m", k=2)[:, :, cb * P:(cb + 1) * P],
                                xc8_cur[kbp][:].rearrange("p (k n) -> p k n", k=2),
                                start=(kbp == 0), stop=(kbp == CB // 2 - 1),
                                perf_mode=DR)
                        for kbp in range(CB // 2):
                            nc.tensor.matmul(
                                i_ps[:],
                                wi_t[kbp][:].rearrange("p (k m) -> p k m", k=2)[:, :, cb * P:(cb + 1) * P],
                                xc8_cur[kbp][:].rearrange("p (k n) -> p k n", k=2),
                                start=(kbp == 0), stop=(kbp == CB // 2 - 1),
                                perf_mode=DR)
                        th_r = sa.tile([P, TT], F16, tag="th_r", bufs=2)
                        act(th_r[:], r_ps[:], AF.Tanh, bias=br_t[:, cb:cb + 1], scale=GS)
                        a_t = sa.tile([P, TT], F32, tag="a_t", bufs=10)
                        act(a_t[:], th_r[:], AF.Exp,
                            bias=c1_t[:, cb:cb + 1], scale=c1_t[:, cb:cb + 1])
                        a_tiles[cb] = a_t
                        th_i = sa.tile([P, TT], F16, tag="th_i", bufs=10)
                        act(th_i[:], i_ps[:], AF.Tanh, bias=bi_t[:, cb:cb + 1], scale=GS)
                        thi_tiles[cb] = th_i
                        nc.vector.tensor_scalar_add(
                            am1_st[cb][:, t0 * TT:(t0 + 1) * TT], a_t[:], -1.0)

                    # -- sqrt batch + gated input + pass-1 scan --
                    for cb in range(CB):
                        w_t = sa.tile([P, TT], F32, tag="w_t", bufs=2)
                        nc.gpsimd.tensor_tensor(w_t[:], a_tiles[cb][:], a_tiles[cb][:], OP.mult)
                        scl = sa.tile([P, TT], F16, tag="scl", bufs=2)
                        # scl = 0.5*sqrt(1-a^2) (0.5 folds the sigmoid of th_i)
                        act(scl[:], w_t[:], AF.Sqrt, scale=-0.25, bias=qtr_t[:, 0:1])
                        v = sa.tile([P, TT], F16, tag="v_t", bufs=2)
                        nc.vector.tensor_tensor(v[:], scl[:], xc_cur[cb][:], OP.mult)
                        bb_sl = bb_st[cb][:, t0 * TT:(t0 + 1) * TT]
                        nc.vector.scalar_tensor_tensor(
                            bb_sl, thi_tiles[cb][:], 1.0, v[:], OP.add, OP.mult)
                        s1 = sa.tile([P, TT], F32, tag="s1", bufs=2)
                        init = 0.0 if t0 == 0 else cy_t[cb][:, 0:1]
                        nc.vector.tensor_tensor_scan(
                            s1[:], a_tiles[cb][:], bb_sl, init, OP.mult, OP.add)
                        if t0 == NT - 1:
                            nc.vector.tensor_copy(hl_sb[:, cb:cb + 1], s1[:, TT - 1:TT])
                        else:
                            nc.vector.tensor_copy(cy_t[cb][:], s1[:, TT - 1:TT])
                    xc_cur, xc8_cur = xc_nxt, xc8_nxt

                # ==== collective: exchange local h_last (inside phase-A scope
                # so it is not queued behind the pool-release syncs) ====
                nc.sync.dma_start(ag_in[:].rearrange("one (cb p) -> p (one cb)", p=P), hl_sb[:])
                nc.gpsimd.collective_compute(
                    "AllGather", OP.bypass,
                    replica_groups=[[0, 1], [2, 3], [4, 5], [6, 7]],
                    ins=[ag_in[:].opt()], outs=[ag_out[:].opt()],
                )
                nc.sync.dma_start(g0[:], ag_out[0:1, :].rearrange("one (cb p) -> p (one cb)", p=P))
                nc.vector.tensor_scalar_mul(init_c[:], g0[:], tm_t[:, 0:1])

            # =========== PHASE B: true scan + out-proj + RMSNorm ===========
            with (
                tc.tile_pool(name="sbufB", bufs=1) as sb,
                tc.tile_pool(name="psumB", bufs=1, space="PSUM") as psb,
            ):
                # a2 = 1 + am1 on ACT (Identity in every table; the first ones
                # run during the collective window)
                a2_t = [[None] * CB for _ in range(NT)]
                for t0 in range(NT):
                    for cb in range(CB):
                        a2 = sb.tile([P, TT], F32, tag="a2", bufs=10)
                        act(a2[:], am1_st[cb][:, t0 * TT:(t0 + 1) * TT],
                            AF.Identity, bias=1.0)
                        a2_t[t0][cb] = a2
                h_prev = [None] * CB
                for t0 in range(NT):
                    h_t = []
                    for cb in range(CB):
                        h = sb.tile([P, TT], F16, tag="h", bufs=17)
                        init = (init_c[:, cb:cb + 1] if t0 == 0
                                else h_prev[cb][:, TT - 1:TT])
                        nc.vector.tensor_tensor_scan(
                            h[:], a2_t[t0][cb][:], bb_st[cb][:, t0 * TT:(t0 + 1) * TT],
                            init, OP.mult, OP.add)
                        h_t.append(h)
                    for cb in range(CB):
                        h_prev[cb] = h_t[cb]
                    for ch in range(TT // P):
                        o_ps = psb.tile([P, D], F32, tag="o_ps", bufs=2)
                        for jh in range(2):
                            for kb in range(CB):
                                nc.tensor.matmul(
                                    o_ps[:, jh * 512:(jh + 1) * 512],
                                    h_t[kb][:, ch * P:(ch + 1) * P],
                                    wo_t[kb][:, jh * 512:(jh + 1) * 512],
                                    start=(kb == 0), stop=(kb == CB - 1))
                        sq0 = sb.tile([P, 512], F32, tag="sq0", bufs=2)
                        ss0 = sb.tile([P, 1], F32, tag="ss0", bufs=2)
                        act(sq0[:], o_ps[:, 0:512], AF.Square, accum_out=ss0[:])
                        sq1 = sb.tile([P, 512], F32, tag="sq1", bufs=2)
                        ss1 = sb.tile([P, 1], F32, tag="ss1", bufs=2)
                        act(sq1[:], o_ps[:, 512:1024], AF.Square, accum_out=ss1[:])
                        ssum = sb.tile([P, 1], F32, tag="ssum", bufs=2)
                        nc.vector.tensor_tensor(ssum[:], ss0[:], ss1[:], OP.add)
                        s = sb.tile([P, 1], F32, tag="s_rms", bufs=2)
                        act(s[:], ssum[:], AF.Sqrt, scale=1.0 / D, bias=eps_t[:, 0:1])
                        rinv = sb.tile([P, 1], F32, tag="rinv", bufs=2)
                        nc.vector.reciprocal(rinv[:], s[:])
                        y_sb = sb.tile([P, D], F32, tag="y_sb", bufs=2)
                        act(y_sb[:, 0:512], o_ps[:, 0:512], AF.Copy, scale=rinv[:, 0:1])
                        act(y_sb[:, 512:1024], o_ps[:, 512:1024], AF.Copy, scale=rinv[:, 0:1])
                        nc.sync.dma_start(
                            y_d[t0 * TT + ch * P: t0 * TT + (ch + 1) * P, :], y_sb[:])

    nc.compile()
    return nc


def kernel(**inputs):
    x = np.asarray(inputs["x"], np.float32)
    conv_w = np.asarray(inputs["conv_w"], np.float32)
    conv_b = np.asarray(inputs["conv_b"], np.float32)
    W_r = np.asarray(inputs["W_r"], np.float32)
    b_r = np.asarray(inputs["b_r"], np.float32)
    W_i = np.asarray(inputs["W_i"], np.float32)
    b_i = np.asarray(inputs["b_i"], np.float32)
    log_a = np.asarray(inputs["log_a"], np.float32)
    W_out = np.asarray(inputs["W_out"], np.float32)
    gamma = np.asarray(inputs["gamma"], np.float32)
    assert x.shape == (B, L, D), x.shape

    if "nc" not in _compiled:
        _compiled["nc"] = _build()
    nc = _compiled["nc"]

    def col(v):
        return np.ascontiguousarray(v.reshape(CB, P).T).astype(np.float32)

    def fp8_pairs(W):
        # [D/2, 2*D] with arr[kbp*128+p, j*D+m] = WS * W.T[(2*kbp+j)*128+p, m]
        wt = (WS * W.T).astype(ml_dtypes.float8_e4m3)
        return np.ascontiguousarray(
            wt.reshape(CB // 2, 2, P, D).transpose(0, 2, 1, 3).reshape(D // 2, 2 * D))

    xT = np.ascontiguousarray(x.transpose(0, 2, 1))            # [B, D, L]
    woT = np.ascontiguousarray((W_out * gamma[:, None]).T).astype(np.float16)
    # diagonal conv-tap blocks: dwk[cb*128+p, k*128+p] = conv_w[cb*128+p, 0, k]
    dwk = np.zeros((CB, P, 4, P), np.float32)
    idx = np.arange(P)
    for cb in range(CB):
        for k in range(4):
            dwk[cb, idx, k, idx] = conv_w[cb * P + idx, 0, k]
    dwk = dwk.reshape(D, 4 * P)
    a_base = 1.0 / (1.0 + np.exp(-log_a.astype(np.float64)))
    c1 = (8.0 * np.log(a_base)).astype(np.float32)
    common = {
        "wr8": fp8_pairs(W_r), "wi8": fp8_pairs(W_i), "woT": woT, "dwk": dwk,
        "br_c": col(0.5 * b_r), "bi_c": col(0.5 * b_i), "cb_c": col(conv_b),
        "c1_c": col(0.5 * c1),
    }
    in_maps = []
    for k in range(8):
        b, th = k // 2, k % 2
        xs = np.zeros((D, TH + 3), np.float32)
        lo = th * TH - 3
        if lo < 0:
            xs[:, 3:] = xT[b, :, 0:TH]
        else:
            xs[:] = xT[b, :, lo:lo + TH + 3]
        m = dict(common)
        m["x_sh"] = xs
        m["tmask"] = np.full((P, 1), float(th), np.float32)
        in_maps.append(m)

    import os
    trace = bool(int(os.environ.get("KERNEL_TRACE", "0")))
    kw = {}
    if trace:
        kw = dict(trace=True, trace_cores=list(range(8)))
    res = bass_utils.run_bass_kernel_spmd(nc, in_maps, core_ids=list(range(8)), **kw)
    _compiled["last_exec_time_ns"] = res.exec_time_ns
    _compiled["last_res"] = res

    out = np.empty((B, L, D), np.float32)
    for k in range(8):
        b, th = k // 2, k % 2
        out[b, th * TH:(th + 1) * TH, :] = res.results[k]["y"]
    return out


# revision 20
# speedup vs baseline: 1.1052x; 1.1052x over previous
"""Trainium2 Bass kernel for nn_CausalRecurrenceLayer.

Sharding: 8 cores = 4 batches x 2 sequence-halves. Channel-major [c, t]
device layout for conv/gates/scan; output projection emits [t, j].

Single-pass design (no DRAM spills): phase A computes conv -> fp8 gate
matmuls (DoubleRow) -> decay am1 (f16, SBUF-resident) and gated input bb
(f16, SBUF-resident) -> local pass-1 scan (only the tail column is kept).
A 4KB AllGather exchanges h_last across sequence-half pairs. Phase B
re-runs the true scan from the received initial state straight out of
SBUF, then out-projection (f16 PE) + RMSNorm.

Self-contained: hardcodes shapes B=4, L=4096, d=1024.
"""
import sys

sys.path.insert(0, "/opt/trn_rl_repo")

import numpy as np
import ml_dtypes

import concourse.bass as bass  # noqa: F401
from concourse.bass import _add_dep_helper
import concourse.tile as tile
from concourse import bacc, mybir
from concourse import bass_utils

F32 = mybir.dt.float32
F32R = mybir.dt.float32r
F16 = mybir.dt.float16
F8 = mybir.dt.float8e4
AF = mybir.ActivationFunctionType
OP = mybir.AluOpType
DR = mybir.MatmulPerfMode.DoubleRow

B, L, D = 4, 4096, 1024
TH = L // 2      # per-core sequence extent
TT = 512         # time tile
NT = TH // TT    # 4
P = 128
CB = D // P      # 8 channel blocks
EPS = 1e-6
XS = 64.0        # fp8 scale on x_conv
WS = 16.0        # fp8 scale on gate weights
GS = 0.5 / (XS * WS)   # tanh pre-scale compensating both

_compiled = {}


def _build():
    nc = bacc.Bacc("TRN2", target_bir_lowering=False, debug=False, num_devices=8)

    x_d = nc.dram_tensor("x_sh", [D, TH + 3], F32R, kind="ExternalInput").ap()
    dw_d = nc.dram_tensor("dwk", [D, 4 * P], F32R, kind="ExternalInput").ap()
    wr_d = nc.dram_tensor("wr8", [D // 2, 2 * D], F8, kind="ExternalInput").ap()
    wi_d = nc.dram_tensor("wi8", [D // 2, 2 * D], F8, kind="ExternalInput").ap()
    wo_d = nc.dram_tensor("woT", [D, D], F16, kind="ExternalInput").ap()
    br_d = nc.dram_tensor("br_c", [P, CB], F32, kind="ExternalInput").ap()   # b_r/2
    bi_d = nc.dram_tensor("bi_c", [P, CB], F32, kind="ExternalInput").ap()   # b_i/2
    cb_d = nc.dram_tensor("cb_c", [P, CB], F32, kind="ExternalInput").ap()   # conv bias
    c1_d = nc.dram_tensor("c1_c", [P, CB], F32, kind="ExternalInput").ap()   # 4*ln(a_base)
    tm_d = nc.dram_tensor("tmask", [P, 1], F32, kind="ExternalInput").ap()
    y_d = nc.dram_tensor("y", [TH, D], F32, kind="ExternalOutput").ap()

    last_act = [None]
    _CHAINED = (AF.Tanh, AF.Exp, AF.Sqrt)

    def act(out, in_, func, **kw):
        ins = nc.scalar.activation(out, in_, func, **kw)
        if func in _CHAINED:
            if last_act[0] is not None:
                _add_dep_helper(ins.ins, last_act[0].ins, reason="act table order")
            last_act[0] = ins
        return ins

    with tile.TileContext(nc) as tc:
        with (
            tc.tile_pool(name="wpool", bufs=1) as wpool,
            tc.tile_pool(name="store", bufs=1) as store,
            tc.tile_pool(name="dram", bufs=1, space="DRAM") as dp,
        ):
            # ---- resident weights / constants ----
            br_t = wpool.tile([P, CB], F32, tag="br")
            nc.sync.dma_start(br_t[:], br_d)
            bi_t = wpool.tile([P, CB], F32, tag="bi")
            nc.sync.dma_start(bi_t[:], bi_d)
            cb_t = wpool.tile([P, CB], F32, tag="cbias")
            nc.sync.dma_start(cb_t[:], cb_d)
            c1_t = wpool.tile([P, CB], F32, tag="c1")
            nc.sync.dma_start(c1_t[:], c1_d)
            tm_t = wpool.tile([P, 1], F32, tag="tm")
            nc.sync.dma_start(tm_t[:], tm_d)
            dw_t, wr_t, wi_t, wo_t = [], [], [], []
            for cb in range(CB):
                t = wpool.tile([P, 4 * P], F32R, tag=f"dw{cb}", name=f"dw{cb}")
                nc.sync.dma_start(t[:], dw_d[cb * P:(cb + 1) * P, :])
                dw_t.append(t)
            for kbp in range(CB // 2):
                t = wpool.tile([P, 2 * D], F8, tag=f"wr{kbp}", name=f"wr{kbp}")
                nc.sync.dma_start(t[:], wr_d[kbp * P:(kbp + 1) * P, :])
                wr_t.append(t)
                t = wpool.tile([P, 2 * D], F8, tag=f"wi{kbp}", name=f"wi{kbp}")
                nc.sync.dma_start(t[:], wi_d[kbp * P:(kbp + 1) * P, :])
                wi_t.append(t)
            for cb in range(CB):
                t = wpool.tile([P, D], F16, tag=f"wo{cb}", name=f"wo{cb}")
                nc.sync.dma_start(t[:], wo_d[cb * P:(cb + 1) * P, :])
                wo_t.append(t)
            eps_t = wpool.tile([P, 1], F32, tag="eps")
            nc.vector.memset(eps_t[:], EPS)
            qtr_t = wpool.tile([P, 1], F32, tag="qtr")
            nc.vector.memset(qtr_t[:], 0.25)
            xsb_t = wpool.tile([P, TT], F16, tag="xsb")
            nc.vector.memset(xsb_t[:], XS)

            # SBUF residents carried from phase A to phase B
            am1_st = [wpool.tile([P, TH], F16, tag=f"am1_{cb}", name=f"am1_{cb}")
                      for cb in range(CB)]
            bb_st = [wpool.tile([P, TH], F16, tag=f"bb_{cb}", name=f"bb_{cb}")
                     for cb in range(CB)]

            hl_sb = store.tile([P, CB], F32, tag="hl")
            cy_t = [store.tile([P, 1], F32, tag=f"cy{cb}", name=f"cy{cb}")
                    for cb in range(CB)]
            g0 = store.tile([P, CB], F32, tag="g0")
            init_c = store.tile([P, CB], F32, tag="init_c")
            ag_in = dp.tile([1, D], F32, tag="ag_in")
            ag_out = dp.tile([2, D], F32, tag="ag_out")

            # =========== PHASE A ===========
            with (
                tc.tile_pool(name="sbufA", bufs=1) as sa,
                tc.tile_pool(name="psumA", bufs=1, space="PSUM") as psa,
            ):
                def conv_loop(t0):
                    # conv on PE + DVE evict + Pool fp8 cast
                    xc_t, xc8_t = [], []
                    for cb in range(CB):
                        xt = sa.tile([P, TT + 3], F32R, tag="xraw", bufs=2)
                        nc.scalar.dma_start(
                            xt[:], x_d[cb * P:(cb + 1) * P, t0 * TT:t0 * TT + TT + 3])
                        xc_ps = psa.tile([P, TT], F32, tag="xc_ps", bufs=2)
                        for k in range(4):
                            nc.tensor.matmul(xc_ps[:], dw_t[cb][:, k * P:(k + 1) * P],
                                             xt[:, k:k + TT], start=(k == 0), stop=(k == 3))
                        xc = sa.tile([P, TT], F16, tag="xc", bufs=18)
                        nc.vector.tensor_scalar_add(xc[:], xc_ps[:], cb_t[:, cb:cb + 1])
                        xc_t.append(xc)
                        if cb % 2 == 0:
                            x8 = sa.tile([P, 2 * TT], F8, tag="xc8", bufs=8)
                            xc8_t.append(x8)
                        nc.gpsimd.tensor_tensor(
                            xc8_t[cb // 2][:, (cb % 2) * TT:(cb % 2 + 1) * TT],
                            xc[:], xsb_t[:], OP.mult)
                    return xc_t, xc8_t

                xc_cur, xc8_cur = conv_loop(0)
                for t0 in range(NT):
                    xc_nxt = xc8_nxt = None
                    if t0 + 1 < NT:
                        xc_nxt, xc8_nxt = conv_loop(t0 + 1)

                    # -- fp8 DoubleRow gate matmuls + tanh/exp (one table set) --
                    a_tiles = [None] * CB
                    thi_tiles = [None] * CB
                    for cb in range(CB):
                        r_ps = psa.tile([P, TT], F32, tag="r_ps", bufs=3)
                        i_ps = psa.tile([P, TT], F32, tag="i_ps", bufs=3)
                        for kbp in range(CB // 2):
                            nc.tensor.matmul(
                                r_ps[:],
                                wr_t[kbp][:].rearrange("p (k m) -> p k m", k=2)[:, :, cb * P:(cb + 1) * P],
                                xc8_cur[kbp][:].rearrange("p (k n) -> p k n", k=2),
                                start=(kbp == 0), stop=(kbp == CB // 2 - 1),
                                perf_mode=DR)
                        for kbp in range(CB // 2):
                            nc.tensor.matmul(
                                i_ps[:],
                                wi_t[kbp][:].rearrange("p (k m) -> p k m", k=2)[:, :, cb * P:(cb + 1) * P],
                                xc8_cur[kbp][:].rearrange("p (k n) -> p k n", k=2),
                                start=(kbp == 0), stop=(kbp == CB // 2 - 1),
                                perf_mode=DR)
                        th_r = sa.tile([P, TT], F16, tag="th_r", bufs=2)
                        act(th_r[:], r_ps[:], AF.Tanh, bias=br_t[:, cb:cb + 1], scale=GS)
                        a_t = sa.tile([P, TT], F32, tag="a_t", bufs=10)
                        act(a_t[:], th_r[:], AF.Exp,
                            bias=c1_t[:, cb:cb + 1], scale=c1_t[:, cb:cb + 1])
                        a_tiles[cb] = a_t
                        th_i = sa.tile([P, TT], F16, tag="th_i", bufs=10)
                        act(th_i[:], i_ps[:], AF.Tanh, bias=bi_t[:, cb:cb + 1], scale=GS)
                        thi_tiles[cb] = th_i
                        nc.vector.tensor_scalar_add(
                            am1_st[cb][:, t0 * TT:(t0 + 1) * TT], a_t[:], -1.0)

                    # -- sqrt batch + gated input + pass-1 scan --
                    for cb in range(CB):
                        w_t = sa.tile([P, TT], F32, tag="w_t", bufs=2)
                        nc.gpsimd.tensor_tensor(w_t[:], a_tiles[cb][:], a_tiles[cb][:], OP.mult)
                        scl = sa.tile([P, TT], F16, tag="scl", bufs=2)
                        # scl = 0.5*sqrt(1-a^2) (0.5 folds the sigmoid of th_i)
                        act(scl[:], w_t[:], AF.Sqrt, scale=-0.25, bias=qtr_t[:, 0:1])
                        v = sa.tile([P, TT], F16, tag="v_t", bufs=2)
                        nc.vector.tensor_tensor(v[:], scl[:], xc_cur[cb][:], OP.mult)
                        bb_sl = bb_st[cb][:, t0 * TT:(t0 + 1) * TT]
                        nc.vector.scalar_tensor_tensor(
                            bb_sl, thi_tiles[cb][:], 1.0, v[:], OP.add, OP.mult)
                        s1 = sa.tile([P, TT], F32, tag="s1", bufs=2)
                        init = 0.0 if t0 == 0 else cy_t[cb][:, 0:1]
                        nc.vector.tensor_tensor_scan(
                            s1[:], a_tiles[cb][:], bb_sl, init, OP.mult, OP.add)
                        if t0 == NT - 1:
                            nc.vector.tensor_copy(hl_sb[:, cb:cb + 1], s1[:, TT - 1:TT])
                        else:
                            nc.vector.tensor_copy(cy_t[cb][:], s1[:, TT - 1:TT])
                    xc_cur, xc8_cur = xc_nxt, xc8_nxt

                # ==== collective: exchange local h_last (inside phase-A scope
                # so it is not queued behind the pool-release syncs) ====
                nc.sync.dma_start(ag_in[:].rearrange("one (cb p) -> p (one cb)", p=P), hl_sb[:])
                nc.gpsimd.collective_compute(
                    "AllGather", OP.bypass,
                    replica_groups=[[0, 1], [2, 3], [4, 5], [6, 7]],
                    ins=[ag_in[:].opt()], outs=[ag_out[:].opt()],
                )
                nc.sync.dma_start(g0[:], ag_out[0:1, :].rearrange("one (cb p) -> p (one cb)", p=P))
                nc.vector.tensor_scalar_mul(init_c[:], g0[:], tm_t[:, 0:1])

            # =========== PHASE B: true scan + out-proj + RMSNorm ===========
            with (
                tc.tile_pool(name="sbufB", bufs=1) as sb,
                tc.tile_pool(name="psumB", bufs=1, space="PSUM") as psb,
            ):
                # a2 = 1 + am1 on ACT (Identity in every table; the first ones
                # run during the collective window)
                a2_t = [[None] * CB for _ in range(NT)]
                for t0 in range(NT):
                    for cb in range(CB):
                        a2 = sb.tile([P, TT], F32, tag="a2", bufs=9)
                        act(a2[:], am1_st[cb][:, t0 * TT:(t0 + 1) * TT],
                            AF.Identity, bias=1.0)
                        a2_t[t0][cb] = a2
                h_prev = [None] * CB
                for t0 in range(NT):
                    h_t = []
                    for cb in range(CB):
                        h = sb.tile([P, TT], F16, tag="h", bufs=26)
                        init = (init_c[:, cb:cb + 1] if t0 == 0
                                else h_prev[cb][:, TT - 1:TT])
                        nc.vector.tensor_tensor_scan(
                            h[:], a2_t[t0][cb][:], bb_st[cb][:, t0 * TT:(t0 + 1) * TT],
                            init, OP.mult, OP.add)
                        h_t.append(h)
                    for cb in range(CB):
                        h_prev[cb] = h_t[cb]
                    for ch in range(TT // P):
                        o_ps = psb.tile([P, D], F32, tag="o_ps", bufs=2)
                        for jh in range(2):
                            for kb in range(CB):
                                nc.tensor.matmul(
                                    o_ps[:, jh * 512:(jh + 1) * 512],
                                    h_t[kb][:, ch * P:(ch + 1) * P],
                                    wo_t[kb][:, jh * 512:(jh + 1) * 512],
                                    start=(kb == 0), stop=(kb == CB - 1))
                        sq0 = sb.tile([P, 512], F32, tag="sq0", bufs=2)
                        ss0 = sb.tile([P, 1], F32, tag="ss0", bufs=2)
                        act(sq0[:], o_ps[:, 0:512], AF.Square, accum_out=ss0[:])
                        sq1 = sb.tile([P, 512], F32, tag="sq1", bufs=2)
                        ss1 = sb.tile([P, 1], F32, tag="ss1", bufs=2)
                        act(sq1[:], o_ps[:, 512:1024], AF.Square, accum_out=ss1[:])
                        ssum = sb.tile([P, 1], F32, tag="ssum", bufs=2)
                        nc.vector.tensor_tensor(ssum[:], ss0[:], ss1[:], OP.add)
                        s = sb.tile([P, 1], F32, tag="s_rms", bufs=2)
                        act(s[:], ssum[:], AF.Sqrt, scale=1.0 / D, bias=eps_t[:, 0:1])
                        rinv = sb.tile([P, 1], F32, tag="rinv", bufs=2)
                        nc.vector.reciprocal(rinv[:], s[:])
                        y_sb = sb.tile([P, D], F32, tag="y_sb", bufs=2)
                        act(y_sb[:, 0:512], o_ps[:, 0:512], AF.Copy, scale=rinv[:, 0:1])
                        act(y_sb[:, 512:1024], o_ps[:, 512:1024], AF.Copy, scale=rinv[:, 0:1])
                        nc.sync.dma_start(
                            y_d[t0 * TT + ch * P: t0 * TT + (ch + 1) * P, :], y_sb[:])

    nc.compile()
    return nc


def kernel(**inputs):
    x = np.asarray(inputs["x"], np.float32)
    conv_w = np.asarray(inputs["conv_w"], np.float32)
    conv_b = np.asarray(inputs["conv_b"], np.float32)
    W_r = np.asarray(inputs["W_r"], np.float32)
    b_r = np.asarray(inputs["b_r"], np.float32)
    W_i = np.asarray(inputs["W_i"], np.float32)
    b_i = np.asarray(inputs["b_i"], np.float32)
    log_a = np.asarray(inputs["log_a"], np.float32)
    W_out = np.asarray(inputs["W_out"], np.float32)
    gamma = np.asarray(inputs["gamma"], np.float32)
    assert x.shape == (B, L, D), x.shape

    if "nc" not in _compiled:
        _compiled["nc"] = _build()
    nc = _compiled["nc"]

    def col(v):
        return np.ascontiguousarray(v.reshape(CB, P).T).astype(np.float32)

    def fp8_pairs(W):
        # [D/2, 2*D] with arr[kbp*128+p, j*D+m] = WS * W.T[(2*kbp+j)*128+p, m]
        wt = (WS * W.T).astype(ml_dtypes.float8_e4m3)
        return np.ascontiguousarray(
            wt.reshape(CB // 2, 2, P, D).transpose(0, 2, 1, 3).reshape(D // 2, 2 * D))

    xT = np.ascontiguousarray(x.transpose(0, 2, 1))            # [B, D, L]
    woT = np.ascontiguousarray((W_out * gamma[:, None]).T).astype(np.float16)
    # diagonal conv-tap blocks: dwk[cb*128+p, k*128+p] = conv_w[cb*128+p, 0, k]
    dwk = np.zeros((CB, P, 4, P), np.float32)
    idx = np.arange(P)
    for cb in range(CB):
        for k in range(4):
            dwk[cb, idx, k, idx] = conv_w[cb * P + idx, 0, k]
    dwk = dwk.reshape(D, 4 * P)
    a_base = 1.0 / (1.0 + np.exp(-log_a.astype(np.float64)))
    c1 = (8.0 * np.log(a_base)).astype(np.float32)
    common = {
        "wr8": fp8_pairs(W_r), "wi8": fp8_pairs(W_i), "woT": woT, "dwk": dwk,
        "br_c": col(0.5 * b_r), "bi_c": col(0.5 * b_i), "cb_c": col(conv_b),
        "c1_c": col(0.5 * c1),
    }
    in_maps = []
    for k in range(8):
        b, th = k // 2, k % 2
        xs = np.zeros((D, TH + 3), np.float32)
        lo = th * TH - 3
        if lo < 0:
            xs[:, 3:] = xT[b, :, 0:TH]
        else:
            xs[:] = xT[b, :, lo:lo + TH + 3]
        m = dict(common)
        m["x_sh"] = xs
        m["tmask"] = np.full((P, 1), float(th), np.float32)
        in_maps.append(m)

    import os
    trace = bool(int(os.environ.get("KERNEL_TRACE", "0")))
    kw = {}
    if trace:
        kw = dict(trace=True, trace_cores=list(range(8)))
    res = bass_utils.run_bass_kernel_spmd(nc, in_maps, core_ids=list(range(8)), **kw)
    _compiled["last_exec_time_ns"] = res.exec_time_ns
    _compiled["last_res"] = res

    out = np.empty((B, L, D), np.float32)
    for k in range(8):
        b, th = k // 2, k % 2
        out[b, th * TH:(th + 1) * TH, :] = res.results[k]["y"]
    return out
